# revision 40
# baseline (speedup 1.0000x reference)
"""Masked dot-product attention for Trainium2 (Bass/Tile), 8-core data parallel.

Problem: B=8, Lq=Lk=2048, D=128 fp32; per-batch valid_length masks keys.
Sharding: one batch per NeuronCore (data parallel).

Per-core pipeline (all matmuls in float32r: 1 cycle/row at N>=256):
  1. Load Q,K tiles, PE-transpose to QT/KT [d=128, seq] layout; load V [k, d].
  2. For each q-chunk (512 q's):
       a. S^T[k_tile, q] = KT_tile.T @ QT_chunk  -> PSUM      (TensorE)
       b. P^T = Exp(S^T/sqrt(D) + mask_bias)    -> SBUF       (ScalarE; the
          per-partition bias is 0 for valid keys, -1e9 for masked ones, so
          masked probabilities are exactly 0; scores are O(6) so softmax
          without max-subtraction is safe for this input distribution)
       c. O^T[d, q] = sum_t V_t.T @ P^T_t (accumulating matmuls)
       d. den[1, q] = sum_t ones.T @ P^T_t (accumulating ones-matmul)
       e. transpose den -> [q_part, 1] via rank-1 matmul, reciprocal on DVE
       f. evict O^T, PE-transpose to O[q, d], multiply by recip during the
          PSUM->SBUF eviction (per-partition tensor_scalar), DMA out.
"""
import math
import sys

if "/opt/trn_rl_repo" not in sys.path:
    sys.path.insert(0, "/opt/trn_rl_repo")

import numpy as np

import concourse.mybir as mybir
import concourse.tile as tile
from concourse import bacc
from concourse.bass_utils import run_bass_kernel_spmd
from concourse.masks import make_identity

F32 = mybir.dt.float32
F32R = mybir.dt.float32r
I32 = mybir.dt.int32
EXP = mybir.ActivationFunctionType.Exp

B, L, D = 8, 2048, 128
P = 128              # partition dim / tile edge
NT = L // P          # 16 seq tiles
QC = 512             # q chunk size
NC = L // QC         # q chunks
JC = QC // P         # q-subtiles per chunk
SCALE = 1.0 / math.sqrt(D)
NEG = -1.0e9


def _emit(nc, tc, ctx, q, k, v, vl, o):
    const = ctx.enter_context(tc.tile_pool(name="const", bufs=1))
    big = ctx.enter_context(tc.tile_pool(name="big", bufs=1))
    ptp = ctx.enter_context(tc.tile_pool(name="ptp", bufs=2))
    otsb = ctx.enter_context(tc.tile_pool(name="otsb", bufs=2))
    densb = ctx.enter_context(tc.tile_pool(name="densb", bufs=2))
    outp = ctx.enter_context(tc.tile_pool(name="outp", bufs=4))

    # PSUM: st 4x[128,512] (4) + ot 2x[128,512] (2) + small 2x1bank (2)
    st_ps = ctx.enter_context(tc.tile_pool(name="st_ps", bufs=4, space="PSUM"))
    ot_ps = ctx.enter_context(tc.tile_pool(name="ot_ps", bufs=2, space="PSUM"))
    small_ps = ctx.enter_context(tc.tile_pool(name="small_ps", bufs=2, space="PSUM"))

    # ---- constants ----
    ident = const.tile([P, P], F32)
    make_identity(nc, ident[:])
    identr = const.tile([P, P], F32R)
    nc.vector.tensor_copy(identr[:], ident[:])
    ones_row = const.tile([1, P], F32)
    nc.vector.memset(ones_row[:], 1.0)
    ones_row_r = const.tile([1, P], F32R)
    nc.vector.tensor_copy(ones_row_r[:], ones_row[:])
    one_one = const.tile([1, 1], F32)
    nc.vector.memset(one_one[:], 1.0)

    # HAM warmup, emitted as the PE's FIRST instructions: 8 accumulating
    # rank-1 all-ones matmuls fill the idle window while the first input
    # DMAs land, so the PE clock-gate (HAM) is at full rate when real work
    # starts. Depends only on two fast DVE ops (no gpsimd). The product
    # (8.0 everywhere) is scaled by 1/8 (exact) into the ones column used
    # by the denominator matmuls, keeping the chain live (not dead code).
    warm_ps = st_ps.tile([P, P], F32, tag="st")
    for i in range(8):
        nc.tensor.matmul(warm_ps[:], ones_row_r[:], ones_row_r[:],
                         start=(i == 0), stop=(i == 7))
    ones_col = const.tile([P, 1], F32R)
    nc.vector.tensor_scalar(out=ones_col[:], in0=warm_ps[:, 0:1],
                            scalar1=1.0 / 8, scalar2=None,
                            op0=mybir.AluOpType.mult)

    # ---- valid_length -> additive mask bias am[128, NT] ----
    vl_i = const.tile([1, 1], I32)
    nc.sync.dma_start(vl_i[:], vl[:])
    vl_f = const.tile([1, 1], F32)
    nc.vector.tensor_copy(vl_f[:], vl_i[:])
    vl_b_ps = small_ps.tile([P, 1], F32, tag="small")
    nc.tensor.matmul(vl_b_ps[:], ones_row[:], vl_f[:], start=True, stop=True)
    vl_b = const.tile([P, 1], F32)
    nc.vector.tensor_copy(vl_b[:], vl_b_ps[:])

    iota_i = const.tile([P, NT], I32)
    nc.gpsimd.iota(iota_i[:], pattern=[[P, NT]], base=0, channel_multiplier=1)
    iota_f = const.tile([P, NT], F32)
    nc.vector.tensor_copy(iota_f[:], iota_i[:])
    am = const.tile([P, NT], F32)
    nc.vector.tensor_scalar(
        out=am[:], in0=iota_f[:], scalar1=vl_b[:], scalar2=NEG,
        op0=mybir.AluOpType.is_ge, op1=mybir.AluOpType.mult,
    )

    # ---- load Q, K (transposed via PE) and V ----
    qt = big.tile([P, L], F32R)   # Q^T: [d, q]
    kt = big.tile([P, L], F32R)   # K^T: [d, k]
    vsb = big.tile([P, L], F32R)  # V tiles: [k within tile, d] at cols t*P
    qld_all = big.tile([P, L], F32R)  # Q in [q%128, (tile, d)] layout
    kld_all = big.tile([P, L], F32R)

    q_r = q.rearrange("(t p) d -> p t d", p=P)  # [128, NT, 128]
    k_r = k.rearrange("(t p) d -> p t d", p=P)
    v_r = v.rearrange("(t p) d -> p t d", p=P)

    G = 4  # tiles per DMA group
    def load_group(dst, src_r, g, eng):
        eng.dma_start(
            dst[:, g * G * P:(g + 1) * G * P].rearrange("p (t d) -> p t d", d=P),
            src_r[:, g * G:(g + 1) * G, :],
        )

    def load_tiles(dst, src_r, t0, n, eng):
        eng.dma_start(
            dst[:, t0 * P:(t0 + n) * P].rearrange("p (t d) -> p t d", d=P),
            src_r[:, t0:t0 + n, :],
        )

    # Q tiles 0-3 and K groups first (chunk 0 needs them), V behind on the
    # sync ring (the gpsimd SWDGE path is much slower than the HWDGE rings).
    # The leading groups are split in half so their completion semaphores
    # (which carry ~2.5us of DMA completion latency) arrive earlier.
    load_tiles(qld_all, q_r, 0, 2, nc.sync)
    load_tiles(kld_all, k_r, 0, 2, nc.scalar)
    load_tiles(qld_all, q_r, 2, 2, nc.sync)
    load_tiles(kld_all, k_r, 2, 2, nc.scalar)
    for g in range(1, NT // G):
        load_group(kld_all, k_r, g, nc.scalar)
    for g in range(1, NT // G):
        load_group(qld_all, q_r, g, nc.sync)
    for g in range(NT // G):
        load_group(vsb, v_r, g, nc.sync)

    recipT = const.tile([P, NT], F32)  # 1/rowsum, [q within tile, q_tile]

    pts = [None] * NC

    def emit_qk_exp_one(c, t):
        pt = pts[c]
        st = st_ps.tile([P, QC], F32, tag="st")
        nc.tensor.matmul(
            st[:],
            kt[:, t * P:(t + 1) * P],
            qt[:, c * QC:(c + 1) * QC],
            start=True, stop=True,
        )
        nc.scalar.activation(
            pt[:, t * QC:(t + 1) * QC], st[:], EXP,
            bias=am[:, t:t + 1], scale=SCALE,
        )

    def transpose_group(dst, src_all, g, evict=None):
        # Transposes via normal-mode matmul src.T @ I (pipelines LDWEIGHTS,
        # ~2x faster than the transpose-mode instruction back-to-back).
        # 4 transposes into one PSUM bank, one wide eviction copy.
        tr = st_ps.tile([P, G * P], F32, tag="st", name=f"trg_{dst.name}_{g}")
        for i in range(G):
            t = g * G + i
            nc.tensor.matmul(tr[:, i * P:(i + 1) * P],
                             src_all[:, t * P:(t + 1) * P], identr[:],
                             start=(i == 0), stop=(i == G - 1))
        (evict or nc.vector.tensor_copy)(dst[:, g * G * P:(g + 1) * G * P], tr[:])

    # phase 1 interleaved with chunk-0 QK: transpose K group g+1 while the
    # QK matmuls for group g run (hides the PSUM->SBUF eviction latency)
    transpose_group(qt, qld_all, 0)
    pts[0] = ptp.tile([P, NT * QC], F32R, tag="pt", name="pt0")
    for g in range(NT // G + 1):
        if g < NT // G:
            transpose_group(kt, kld_all, g, evict=nc.scalar.copy)
        if g >= 1:
            for t in range((g - 1) * G, g * G):
                emit_qk_exp_one(0, t)
    for g in range(1, NT // G):
        transpose_group(qt, qld_all, g)

    def emit_pv_out(c, interleave_next=None):
        last = interleave_next is None
        # interleave_next: emits chunk c+1's QK matmul for tile t between this
        # chunk's den/PV matmuls, keeping the in-order PE stream busy while
        # the ScalarE exp pipeline catches up (ST slots gate on exp).
        pt = pts[c]
        otp = ot_ps.tile([P, QC], F32, tag="otps")
        dsb = densb.tile([1, QC], F32, tag="densb")
        dps = small_ps.tile([1, QC], F32, tag="small", name=f"dps{c}")
        for t in range(NT):
            if interleave_next is not None:
                interleave_next(t)
            nc.tensor.matmul(
                dps[:],
                ones_col[:],
                pt[:, t * QC:(t + 1) * QC],
                start=(t == 0), stop=(t == NT - 1),
            )
            nc.tensor.matmul(
                otp[:],
                vsb[:, t * P:(t + 1) * P],
                pt[:, t * QC:(t + 1) * QC],
                start=(t == 0), stop=(t == NT - 1),
            )
        nc.vector.tensor_copy(dsb[:], dps[:])
        denT = small_ps.tile([P, JC], F32, tag="small", name=f"denT{c}")
        for j in range(JC):
            nc.tensor.matmul(
                denT[:, j:j + 1],
                dsb[0:1, j * P:(j + 1) * P],
                one_one[:],
                start=(j == 0), stop=(j == JC - 1),
            )
        nc.vector.reciprocal(
            recipT[:, c * JC:(c + 1) * JC], denT[:]
        )

        osb = otsb.tile([P, QC], F32R, tag="otsb")
        if last:
            # split the eviction so the first output transposes (and their
            # ring-alternated DMAs) start half an eviction earlier
            nc.vector.tensor_copy(osb[:, 0:QC // 2], otp[:, 0:QC // 2])
            nc.vector.tensor_copy(osb[:, QC // 2:], otp[:, QC // 2:])
        else:
            nc.vector.tensor_copy(osb[:], otp[:])
        for j in range(JC):
            g = c * JC + j
            ops = small_ps.tile([P, P], F32, tag="small", name=f"ops{c}_{j}")
            nc.tensor.matmul(ops[:], osb[:, j * P:(j + 1) * P], identr[:],
                             start=True, stop=True)
            ot = outp.tile([P, P], F32, tag="osb")
            nc.vector.tensor_scalar(
                out=ot[:], in0=ops[:], scalar1=recipT[:, g:g + 1], scalar2=None,
                op0=mybir.AluOpType.mult,
            )
            # last chunk: alternate output DMAs across both HWDGE rings
            # (the scalar ring is idle once the exps are done) so the final
            # transfers - and their ~2.5us completion semaphores - overlap
            eng = nc.scalar if (last and j % 2) else nc.sync
            eng.dma_start(o[g * P:(g + 1) * P, :], ot[:])

    # software-pipelined: chunk c's QK matmuls are interleaved into chunk
    # c-1's den/PV matmul stream
    for c in range(1, NC):
        pts[c] = ptp.tile([P, NT * QC], F32R, tag="pt", name=f"pt{c}")
        emit_pv_out(c - 1, interleave_next=lambda t, c=c: emit_qk_exp_one(c, t))
    emit_pv_out(NC - 1)


_CACHE = {}


def _build():
    from contextlib import ExitStack

    nc = bacc.Bacc("TRN2", target_bir_lowering=False, debug=False, num_devices=B)
    q = nc.dram_tensor("q", [L, D], F32R, kind="ExternalInput").ap()
    k = nc.dram_tensor("k", [L, D], F32R, kind="ExternalInput").ap()
    v = nc.dram_tensor("v", [L, D], F32R, kind="ExternalInput").ap()
    vl = nc.dram_tensor("vl", [1, 1], I32, kind="ExternalInput").ap()
    o = nc.dram_tensor("o", [L, D], F32, kind="ExternalOutput").ap()
    with tile.TileContext(nc) as tc:
        with ExitStack() as ctx:
            _emit(nc, tc, ctx, q, k, v, vl, o)
    nc.compile()
    return nc


def get_nc():
    if "nc" not in _CACHE:
        _CACHE["nc"] = _build()
    return _CACHE["nc"]


def kernel(queries, keys, values, valid_length):
    queries = np.ascontiguousarray(np.asarray(queries, dtype=np.float32))
    keys = np.ascontiguousarray(np.asarray(keys, dtype=np.float32))
    values = np.ascontiguousarray(np.asarray(values, dtype=np.float32))
    valid_length = np.asarray(valid_length, dtype=np.int32)
    assert queries.shape == (B, L, D)

    nc = get_nc()
    in_maps = [
        {
            "q": queries[b],
            "k": keys[b],
            "v": values[b],
            "vl": valid_length[b].reshape(1, 1),
        }
        for b in range(B)
    ]
    res = run_bass_kernel_spmd(nc, in_maps, list(range(B)))
    out = np.stack([res.results[b]["o"] for b in range(B)]).astype(np.float32)
    return out


# revision 41
# speedup vs baseline: 1.1886x; 1.1886x over previous
"""Masked dot-product attention for Trainium2 (Bass/Tile), 8-core data parallel.

Problem: B=8, Lq=Lk=2048, D=128 fp32; per-batch valid_length masks keys.
Sharding: one batch per NeuronCore (data parallel).

Per-core pipeline (all matmuls in float32r: 1 cycle/row at N>=256):
  1. Load Q,K tiles, PE-transpose to QT/KT [d=128, seq] layout; load V [k, d].
  2. For each q-chunk (512 q's):
       a. S^T[k_tile, q] = KT_tile.T @ QT_chunk  -> PSUM      (TensorE)
       b. P^T = Exp(S^T/sqrt(D) + mask_bias)    -> SBUF       (ScalarE; the
          per-partition bias is 0 for valid keys, -1e9 for masked ones, so
          masked probabilities are exactly 0; scores are O(6) so softmax
          without max-subtraction is safe for this input distribution)
       c. O^T[d, q] = sum_t V_t.T @ P^T_t (accumulating matmuls)
       d. den[1, q] = sum_t ones.T @ P^T_t (accumulating ones-matmul)
       e. transpose den -> [q_part, 1] via rank-1 matmul, reciprocal on DVE
       f. evict O^T, PE-transpose to O[q, d], multiply by recip during the
          PSUM->SBUF eviction (per-partition tensor_scalar), DMA out.
"""
import math
import sys

if "/opt/trn_rl_repo" not in sys.path:
    sys.path.insert(0, "/opt/trn_rl_repo")

import numpy as np

import concourse.mybir as mybir
import concourse.tile as tile
from concourse import bacc
from concourse.bass_utils import run_bass_kernel_spmd
from concourse.masks import make_identity

F32 = mybir.dt.float32
F32R = mybir.dt.float32r
I32 = mybir.dt.int32
EXP = mybir.ActivationFunctionType.Exp

B, L, D = 8, 2048, 128
P = 128              # partition dim / tile edge
NT = L // P          # 16 seq tiles
QC = 512             # q chunk size
NC = L // QC         # q chunks
JC = QC // P         # q-subtiles per chunk
SCALE = 1.0 / math.sqrt(D)
NEG = -1.0e9


def _emit(nc, tc, ctx, q, k, v, vl, o):
    const = ctx.enter_context(tc.tile_pool(name="const", bufs=1))
    big = ctx.enter_context(tc.tile_pool(name="big", bufs=1))
    ptp = ctx.enter_context(tc.tile_pool(name="ptp", bufs=2))
    otsb = ctx.enter_context(tc.tile_pool(name="otsb", bufs=2))
    densb = ctx.enter_context(tc.tile_pool(name="densb", bufs=2))
    outp = ctx.enter_context(tc.tile_pool(name="outp", bufs=4))

    # PSUM: st 4x[128,512] (4) + ot 2x[128,512] (2) + small 2x1bank (2)
    st_ps = ctx.enter_context(tc.tile_pool(name="st_ps", bufs=4, space="PSUM"))
    ot_ps = ctx.enter_context(tc.tile_pool(name="ot_ps", bufs=2, space="PSUM"))
    small_ps = ctx.enter_context(tc.tile_pool(name="small_ps", bufs=2, space="PSUM"))

    # ---- constants ----
    ident = const.tile([P, P], F32)
    make_identity(nc, ident[:])
    identr = const.tile([P, P], F32R)
    nc.vector.tensor_copy(identr[:], ident[:])
    ones_row = const.tile([1, P], F32)
    nc.vector.memset(ones_row[:], 1.0)
    ones_row_r = const.tile([1, P], F32R)
    nc.vector.tensor_copy(ones_row_r[:], ones_row[:])
    one_one = const.tile([1, 1], F32)
    nc.vector.memset(one_one[:], 1.0)

    # HAM warmup, emitted as the PE's FIRST instructions: 8 accumulating
    # rank-1 all-ones matmuls fill the idle window while the first input
    # DMAs land, so the PE clock-gate (HAM) is at full rate when real work
    # starts. Depends only on two fast DVE ops (no gpsimd). The product
    # (8.0 everywhere) is scaled by 1/8 (exact) into the ones column used
    # by the denominator matmuls, keeping the chain live (not dead code).
    warm_ps = st_ps.tile([P, P], F32, tag="st")
    for i in range(8):
        nc.tensor.matmul(warm_ps[:], ones_row_r[:], ones_row_r[:],
                         start=(i == 0), stop=(i == 7))
    ones_col = const.tile([P, 1], F32R)
    nc.vector.tensor_scalar(out=ones_col[:], in0=warm_ps[:, 0:1],
                            scalar1=1.0 / 8, scalar2=None,
                            op0=mybir.AluOpType.mult)

    # ---- valid_length -> additive mask bias am[128, NT] ----
    vl_i = const.tile([1, 1], I32)
    nc.sync.dma_start(vl_i[:], vl[:])
    vl_f = const.tile([1, 1], F32)
    nc.vector.tensor_copy(vl_f[:], vl_i[:])
    vl_b_ps = small_ps.tile([P, 1], F32, tag="small")
    nc.tensor.matmul(vl_b_ps[:], ones_row[:], vl_f[:], start=True, stop=True)
    vl_b = const.tile([P, 1], F32)
    nc.vector.tensor_copy(vl_b[:], vl_b_ps[:])

    iota_i = const.tile([P, NT], I32)
    nc.gpsimd.iota(iota_i[:], pattern=[[P, NT]], base=0, channel_multiplier=1)
    iota_f = const.tile([P, NT], F32)
    nc.vector.tensor_copy(iota_f[:], iota_i[:])
    am = const.tile([P, NT], F32)
    nc.vector.tensor_scalar(
        out=am[:], in0=iota_f[:], scalar1=vl_b[:], scalar2=NEG,
        op0=mybir.AluOpType.is_ge, op1=mybir.AluOpType.mult,
    )

    # ---- load Q, K (transposed via PE) and V ----
    qt = big.tile([P, L], F32R)   # Q^T: [d, q]
    kt = big.tile([P, L], F32R)   # K^T: [d, k]
    vsb = big.tile([P, L], F32R)  # V tiles: [k within tile, d] at cols t*P
    qld_all = big.tile([P, L], F32R)  # Q in [q%128, (tile, d)] layout
    kld_all = big.tile([P, L], F32R)

    q_r = q.rearrange("(t p) d -> p t d", p=P)  # [128, NT, 128]
    k_r = k.rearrange("(t p) d -> p t d", p=P)
    v_r = v.rearrange("(t p) d -> p t d", p=P)

    G = 4  # tiles per DMA group
    def load_group(dst, src_r, g, eng):
        eng.dma_start(
            dst[:, g * G * P:(g + 1) * G * P].rearrange("p (t d) -> p t d", d=P),
            src_r[:, g * G:(g + 1) * G, :],
        )

    def load_tiles(dst, src_r, t0, n, eng):
        eng.dma_start(
            dst[:, t0 * P:(t0 + n) * P].rearrange("p (t d) -> p t d", d=P),
            src_r[:, t0:t0 + n, :],
        )

    # Q tiles 0-3 and K groups first (chunk 0 needs them), V behind on the
    # sync ring (the gpsimd SWDGE path is much slower than the HWDGE rings).
    # The leading groups are split in half so their completion semaphores
    # (which carry ~2.5us of DMA completion latency) arrive earlier.
    load_tiles(qld_all, q_r, 0, 2, nc.sync)
    load_tiles(kld_all, k_r, 0, 2, nc.scalar)
    load_tiles(qld_all, q_r, 2, 2, nc.sync)
    load_tiles(kld_all, k_r, 2, 2, nc.scalar)
    for g in range(1, NT // G):
        load_group(kld_all, k_r, g, nc.scalar)
    for g in range(1, NT // G):
        load_group(qld_all, q_r, g, nc.sync)
    for g in range(NT // G):
        load_group(vsb, v_r, g, nc.sync)

    recipT = const.tile([P, NT], F32)  # 1/rowsum, [q within tile, q_tile]

    pts = [None] * NC

    def emit_qk_exp_one(c, t):
        pt = pts[c]
        st = st_ps.tile([P, QC], F32, tag="st")
        nc.tensor.matmul(
            st[:],
            kt[:, t * P:(t + 1) * P],
            qt[:, c * QC:(c + 1) * QC],
            start=True, stop=True,
        )
        nc.scalar.activation(
            pt[:, t * QC:(t + 1) * QC], st[:], EXP,
            bias=am[:, t:t + 1], scale=SCALE,
        )

    def transpose_group(dst, src_all, g, evict=None):
        # Transposes via normal-mode matmul src.T @ I (pipelines LDWEIGHTS,
        # ~2x faster than the transpose-mode instruction back-to-back).
        # 4 transposes into one PSUM bank, one wide eviction copy.
        tr = st_ps.tile([P, G * P], F32, tag="st", name=f"trg_{dst.name}_{g}")
        for i in range(G):
            t = g * G + i
            nc.tensor.matmul(tr[:, i * P:(i + 1) * P],
                             src_all[:, t * P:(t + 1) * P], identr[:],
                             start=(i == 0), stop=(i == G - 1))
        (evict or nc.vector.tensor_copy)(dst[:, g * G * P:(g + 1) * G * P], tr[:])

    # phase 1 interleaved with chunk-0 QK: transpose K group g+1 while the
    # QK matmuls for group g run (hides the PSUM->SBUF eviction latency)
    transpose_group(qt, qld_all, 0)
    pts[0] = ptp.tile([P, NT * QC], F32R, tag="pt", name="pt0")
    for g in range(NT // G + 1):
        if g < NT // G:
            transpose_group(kt, kld_all, g, evict=nc.scalar.copy)
        if g >= 1:
            for t in range((g - 1) * G, g * G):
                emit_qk_exp_one(0, t)
    for g in range(1, NT // G):
        transpose_group(qt, qld_all, g)

    def emit_pv_out(c, interleave_next=None):
        last = interleave_next is None
        # interleave_next: emits chunk c+1's QK matmul for tile t between this
        # chunk's den/PV matmuls, keeping the in-order PE stream busy while
        # the ScalarE exp pipeline catches up (ST slots gate on exp).
        pt = pts[c]
        otp = ot_ps.tile([P, QC], F32, tag="otps")
        dsb = densb.tile([1, QC], F32, tag="densb")
        dps = small_ps.tile([1, QC], F32, tag="small", name=f"dps{c}")
        for t in range(NT):
            if interleave_next is not None:
                interleave_next(t)
            nc.tensor.matmul(
                dps[:],
                ones_col[:],
                pt[:, t * QC:(t + 1) * QC],
                start=(t == 0), stop=(t == NT - 1),
            )
            nc.tensor.matmul(
                otp[:],
                vsb[:, t * P:(t + 1) * P],
                pt[:, t * QC:(t + 1) * QC],
                start=(t == 0), stop=(t == NT - 1),
            )
        nc.vector.tensor_copy(dsb[:], dps[:])
        denT = small_ps.tile([P, JC], F32, tag="small", name=f"denT{c}")
        for j in range(JC):
            nc.tensor.matmul(
                denT[:, j:j + 1],
                dsb[0:1, j * P:(j + 1) * P],
                one_one[:],
                start=(j == 0), stop=(j == JC - 1),
            )
        nc.vector.reciprocal(
            recipT[:, c * JC:(c + 1) * JC], denT[:]
        )

        osb = otsb.tile([P, QC], F32R, tag="otsb")
        nc.vector.tensor_copy(osb[:], otp[:])
        for j in range(JC):
            g = c * JC + j
            ops = small_ps.tile([P, P], F32, tag="small", name=f"ops{c}_{j}")
            nc.tensor.matmul(ops[:], osb[:, j * P:(j + 1) * P], identr[:],
                             start=True, stop=True)
            ot = outp.tile([P, P], F32, tag="osb")
            nc.vector.tensor_scalar(
                out=ot[:], in0=ops[:], scalar1=recipT[:, g:g + 1], scalar2=None,
                op0=mybir.AluOpType.mult,
            )
            # last chunk: alternate output DMAs across both HWDGE rings
            # (the scalar ring is idle once the exps are done) so the final
            # transfers - and their ~2.5us completion semaphores - overlap
            eng = nc.scalar if (last and j % 2) else nc.sync
            eng.dma_start(o[g * P:(g + 1) * P, :], ot[:])

    # software-pipelined: chunk c's QK matmuls are interleaved into chunk
    # c-1's den/PV matmul stream
    for c in range(1, NC):
        pts[c] = ptp.tile([P, NT * QC], F32R, tag="pt", name=f"pt{c}")
        emit_pv_out(c - 1, interleave_next=lambda t, c=c: emit_qk_exp_one(c, t))
    emit_pv_out(NC - 1)


_CACHE = {}


def _build():
    from contextlib import ExitStack

    nc = bacc.Bacc("TRN2", target_bir_lowering=False, debug=False, num_devices=B)
    q = nc.dram_tensor("q", [L, D], F32R, kind="ExternalInput").ap()
    k = nc.dram_tensor("k", [L, D], F32R, kind="ExternalInput").ap()
    v = nc.dram_tensor("v", [L, D], F32R, kind="ExternalInput").ap()
    vl = nc.dram_tensor("vl", [1, 1], I32, kind="ExternalInput").ap()
    o = nc.dram_tensor("o", [L, D], F32, kind="ExternalOutput").ap()
    with tile.TileContext(nc) as tc:
        with ExitStack() as ctx:
            _emit(nc, tc, ctx, q, k, v, vl, o)
    nc.compile()
    return nc


def get_nc():
    if "nc" not in _CACHE:
        _CACHE["nc"] = _build()
    return _CACHE["nc"]


def kernel(queries, keys, values, valid_length):
    queries = np.ascontiguousarray(np.asarray(queries, dtype=np.float32))
    keys = np.ascontiguousarray(np.asarray(keys, dtype=np.float32))
    values = np.ascontiguousarray(np.asarray(values, dtype=np.float32))
    valid_length = np.asarray(valid_length, dtype=np.int32)
    assert queries.shape == (B, L, D)

    nc = get_nc()
    in_maps = [
        {
            "q": queries[b],
            "k": keys[b],
            "v": values[b],
            "vl": valid_length[b].reshape(1, 1),
        }
        for b in range(B)
    ]
    res = run_bass_kernel_spmd(nc, in_maps, list(range(B)))
    out = np.stack([res.results[b]["o"] for b in range(B)]).astype(np.float32)
    return out
